# revision 16
# baseline (speedup 1.0000x reference)
"""Trainium2 Bass kernel for a 2-layer GCN (PyG GCNConv semantics).

Strategy (8 NeuronCores, SPMD, full I/O):
  - Host folds symmetric deg^-1/2 normalization + edge weight into one
    per-edge scalar w~ = dinv[src]*w*dinv[dst]; self-loops become one
    extra "tile" per dst block (sequential z load, w~ = dinv^2).
  - Destinations partitioned contiguously: 8 cores x 98 blocks x 128.
    Sources split into 4 groups so int16 indices work with dma_gather;
    the group windows are 32768 rows (int16 range) vs 25088-row spacing,
    so groups 1-3 can shed overflow edges to the previous group. Static
    per-block tile caps (6,4,4,4) then hold w.h.p., cutting gather
    padding vs. a uniform cap. Cells are padded with dummy index-0 rows
    (weight 0) so gathers have NO -1 skips and NO runtime counts, which
    lets 7 blocks share one dma_gather call per group (amortizes the
    ~1us SWDGE descriptor-generation cost on the Pool engine that
    dominated the previous version).
  - The one-hot scatter matrices are built ON DEVICE from a compact
    (slot, w) stream (4 bytes/edge-slot instead of 256): per block, two
    wide DVE ops over [128, 19*128] fp16:
        S = (iota == slot_bcast) * w_bcast
    using step-0 broadcast access patterns; this removes the 67MB/layer
    one-hot stream from DRAM that previously half-filled the DMA bus.
  - Per dst block: PSUM aggT[f, n] += G_t.T @ S_t over 19 tiles
    (TensorE fp16, fp32 accum), then out = relu(aggT.T @ W + b).
  - Two launches (one per GCN layer) of the same compiled program.
"""

import os
from contextlib import ExitStack

import numpy as np

import concourse.bacc as bacc
import concourse.bass as bass
import concourse.mybir as mybir
import concourse.tile as tile
from concourse.tile import add_dep_helper
from concourse import bass_utils

P = 128
D = 128
NCORES = 8
NGROUP = 4
N_NODES = 100000
NB_PER_CORE = 98
SHARD = NB_PER_CORE * P          # 12544
N_PAD = SHARD * NCORES           # 100352
GROWS = N_PAD // NGROUP          # 25088 group spacing
WINDOW = 1 << 15                 # 32768 int16-addressable rows per group
ZROWS = (NGROUP - 1) * GROWS + WINDOW   # padded z rows: 108032
KBLK = 7                         # dst blocks per dma_gather call
NCALL = NB_PER_CORE // KBLK      # 14
CAPS = (5, 4, 4, 4)              # gather tiles per (block, group)
CAPS_FALLBACK = (6, 4, 4, 4)
PREP = bool(int(os.environ.get("GCN_PREP", "0")))
SBUILD_TILE = bool(int(os.environ.get("GCN_SBUILD_TILE", "0")))

_nc_cache = {}


def build_nc(caps, prep):
    dt = mybir.dt
    TB = sum(caps)
    TB1 = TB + 1
    nblk = NB_PER_CORE
    ixcols = KBLK * TB * 8                 # idx cols per call (16-wrapped)
    swcols = KBLK * 2 * TB1                # slot+w cols per call
    nc = bacc.Bacc(
        "TRN2",
        target_bir_lowering=False,
        debug=False,
        enable_asserts=False,
        num_devices=1,
        num_swdge_queues=4,
    )
    zt = nc.dram_tensor("zt", [ZROWS, D], dt.float16, kind="ExternalInput")
    ixd = nc.dram_tensor("ixd", [NCALL, P, ixcols], dt.int16,
                         kind="ExternalInput")
    swd = nc.dram_tensor("swd", [NCALL, P, swcols], dt.float16,
                         kind="ExternalInput")
    zself = nc.dram_tensor("zself", [nblk * P, D], dt.float16,
                           kind="ExternalInput")
    iot = nc.dram_tensor("iot", [P, TB1 * P], dt.float16, kind="ExternalInput")
    cnt = nc.dram_tensor("cnt", [1, nblk * NGROUP], dt.int32,
                         kind="ExternalInput")
    wt = nc.dram_tensor("wt", [D, D], dt.float16, kind="ExternalInput")
    brow = nc.dram_tensor("brow", [1, D], dt.float16, kind="ExternalInput")
    out = nc.dram_tensor("out", [nblk * P, D], dt.float32,
                         kind="ExternalOutput")

    with tile.TileContext(nc) as tc, ExitStack() as ctx:
        const = ctx.enter_context(tc.tile_pool(name="const", bufs=1))
        meta = ctx.enter_context(tc.tile_pool(name="meta", bufs=3))
        swp = ctx.enter_context(tc.tile_pool(name="swp", bufs=3))
        zsp = ctx.enter_context(tc.tile_pool(name="zsp", bufs=3))
        gpools = [
            ctx.enter_context(tc.tile_pool(name=f"g{g}", bufs=4))
            for g in range(NGROUP)
        ]
        spool = ctx.enter_context(tc.tile_pool(name="s", bufs=4))
        apool = ctx.enter_context(tc.tile_pool(name="agg", bufs=3))
        opool = ctx.enter_context(tc.tile_pool(name="o", bufs=2))
        ppool = ctx.enter_context(tc.tile_pool(name="ps", bufs=3, space="PSUM"))
        p2pool = ctx.enter_context(tc.tile_pool(name="ps2", bufs=2,
                                                space="PSUM"))

        w_t = const.tile([D, D], dt.float16)
        nc.sync.dma_start(out=w_t[:], in_=wt[:])
        b_t = const.tile([1, D], dt.float16)
        nc.sync.dma_start(out=b_t[:], in_=brow[:])
        ones_t = const.tile([1, P], dt.float16)
        nc.vector.memset(ones_t[:], 1.0)
        iota_t = const.tile([P, TB1 * P], dt.float16)
        nc.sync.dma_start(out=iota_t[:], in_=iot[:])
        iota3 = iota_t[:].rearrange("p (t n) -> p t n", n=P)
        cnt_t = const.tile([1, nblk * NGROUP], dt.int32)
        nc.sync.dma_start(out=cnt_t[:], in_=cnt[:])
        GBUFS = 4
        prev_gather = None

        for c in range(NCALL):
            ix = meta.tile([P, ixcols], dt.int16, tag="ix")
            nc.sync.dma_start(out=ix[:], in_=ixd[c])
            sw_t = swp.tile([P, swcols], dt.float16, tag="sw")
            nc.scalar.dma_start(out=sw_t[:], in_=swd[c])
            zs = zsp.tile([P, KBLK * D], dt.float16, tag="zs")
            nc.sync.dma_start(
                out=zs[:].rearrange("p (k d) -> p k d", d=D),
                in_=zself[c * KBLK * P:(c + 1) * KBLK * P, :].rearrange(
                    "(k p) d -> p k d", p=P),
            )

            gcol0 = [0] * NGROUP
            acc = 0
            for g in range(NGROUP):
                gcol0[g] = acc
                acc += KBLK * caps[g] * 8

            ost = opool.tile([P, KBLK * D], dt.float32, tag="o")
            for k in range(KBLK):
                b = c * KBLK + k
                regs = [nc.gpsimd.alloc_register(f"cnt_{b}_{g}")
                        for g in range(NGROUP)]
                ld = nc.gpsimd.reg_load(
                    regs, cnt_t[0:1, b * NGROUP:(b + 1) * NGROUP])
                if prev_gather is not None:
                    add_dep_helper(ld.ins, prev_gather.ins, sync=False,
                                   reason="limit cnt register liveness")
                g_ws = []
                for g in range(NGROUP):
                    g_w = gpools[g].tile([P, caps[g] * P], dt.float16,
                                         tag=f"G{g}")
                    if b < GBUFS:
                        nc.vector.memset(g_w[:], 0.0)
                    o0 = gcol0[g] + k * caps[g] * 8
                    prev_gather = nc.gpsimd.dma_gather(
                        out_ap=g_w[:].rearrange("p (j n) -> p j n", n=P),
                        in_ap=zt[g * GROWS:g * GROWS + WINDOW, :],
                        idxs_ap=ix[:, o0:o0 + caps[g] * 8],
                        num_idxs=caps[g] * P,
                        num_idxs_reg=regs[g],
                        elem_size=P,
                        queue_num=g,
                        single_packet=False,
                    )
                    g_ws.append(g_w)
                s_t = spool.tile([P, TB1 * P], dt.float16, tag="S")
                s3 = s_t[:].rearrange("p (t n) -> p t n", n=P)
                slot_b = sw_t[:, k * 2 * TB1:k * 2 * TB1 + TB1, None] \
                    .broadcast_to((P, TB1, P))
                w_b = sw_t[:, k * 2 * TB1 + TB1:(k + 1) * 2 * TB1, None] \
                    .broadcast_to((P, TB1, P))
                nc.vector.tensor_tensor(
                    out=s3, in0=iota3, in1=slot_b,
                    op=mybir.AluOpType.is_equal)
                eng = nc.gpsimd if (k % 2 == 0) else nc.vector
                eng.tensor_tensor(
                    out=s3, in0=s3, in1=w_b, op=mybir.AluOpType.mult)

                psum = ppool.tile([P, P], dt.float32, tag="psA")
                t = 0
                for g in range(NGROUP):
                    for j in range(caps[g]):
                        nc.tensor.matmul(
                            out=psum[:],
                            lhsT=g_ws[g][:, j * P:(j + 1) * P],
                            rhs=s_t[:, t * P:(t + 1) * P],
                            start=(t == 0),
                            stop=False,
                        )
                        t += 1
                nc.tensor.matmul(
                    out=psum[:],
                    lhsT=zs[:, k * D:(k + 1) * D],
                    rhs=s_t[:, TB * P:TB1 * P],
                    start=False, stop=True,
                )

                agg_t = apool.tile([P, P], dt.float16, tag="aggT")
                nc.scalar.activation(out=agg_t[:], in_=psum[:],
                                     func=mybir.ActivationFunctionType.Copy)

                psum2 = p2pool.tile([P, D], dt.float32, tag="psB")
                nc.tensor.matmul(out=psum2[:], lhsT=agg_t[:], rhs=w_t[:],
                                 start=True, stop=False)
                nc.tensor.matmul(out=psum2[:], lhsT=ones_t[:], rhs=b_t[:],
                                 start=False, stop=True)
                nc.scalar.activation(out=ost[:, k * D:(k + 1) * D],
                                     in_=psum2[:],
                                     func=mybir.ActivationFunctionType.Relu)

            nc.sync.dma_start(
                out=out[c * KBLK * P:(c + 1) * KBLK * P, :].rearrange(
                    "(k p) d -> p k d", p=P),
                in_=ost[:].rearrange("p (k d) -> p k d", d=D),
            )

    nc.compile()
    return nc


def preprocess(src, dst, ew, capst):
    """Host-side: edge->group assignment with overflow cascade, padded
    per-cell gather index streams, and compact (slot, w) scatter metadata.

    Returns (ixd, swd) with
      ixd: [NCORES, NCALL, P, KBLK*TB*8] int16 wrapped gather indices
      swd: [NCORES, NCALL, P, KBLK*2*TB1] fp16 slot/weight columns
    """
    TB = sum(capst)
    TB1 = TB + 1
    TBASE = (0, capst[0], capst[0] + capst[1], capst[0] + capst[1] + capst[2])
    nblocks = NCORES * NB_PER_CORE
    deg = np.bincount(dst, weights=ew.astype(np.float64),
                      minlength=N_NODES) + 1.0
    dinv = (1.0 / np.sqrt(deg)).astype(np.float32)
    wtil = (dinv[src] * ew.astype(np.float32) * dinv[dst]).astype(np.float32)
    wself = np.zeros(N_PAD, np.float32)
    wself[:N_NODES] = dinv * dinv

    blk = (dst // P).astype(np.int64)
    ng = (src // GROWS).astype(np.int64)
    zone = (src % GROWS) < (WINDOW - GROWS)
    elig = zone & (ng >= 1)

    cellng = blk * NGROUP + ng
    n = np.bincount(cellng, minlength=nblocks * NGROUP) \
        .reshape(nblocks, NGROUP)
    e = np.bincount(cellng[elig], minlength=nblocks * NGROUP) \
        .reshape(nblocks, NGROUP)

    caps = np.array(capst) * P
    shed = np.zeros((nblocks, NGROUP), np.int64)
    load = n[:, 3]
    for g in (3, 2, 1):
        shed[:, g] = np.maximum(load - caps[g], 0)
        if not (shed[:, g] <= e[:, g]).all():
            raise RuntimeError("group shed infeasible; raise CAPS")
        load = n[:, g - 1] + shed[:, g]
    if not (load <= caps[0]).all():
        raise RuntimeError("group 0 overflow; raise CAPS")

    # shed the smallest-src eligible edges of each (block, group) cell
    order = np.lexsort((src, ~elig, cellng))
    starts = np.zeros(nblocks * NGROUP, np.int64)
    counts = n.reshape(-1)
    np.cumsum(counts[:-1], out=starts[1:])
    rank = np.arange(len(src)) - starts[cellng[order]]
    shed_sorted = rank < shed.reshape(-1)[cellng[order]]
    ag = ng.copy()
    ag[order[shed_sorted]] -= 1

    # final cells, sorted by (cell, src); position within cell
    cell = blk * NGROUP + ag
    order2 = np.lexsort((src, cell))
    cell_s = cell[order2]
    src_s = src[order2]
    dst_s = dst[order2]
    w_s = wtil[order2]
    counts2 = np.bincount(cell_s, minlength=nblocks * NGROUP)
    starts2 = np.zeros(nblocks * NGROUP, np.int64)
    np.cumsum(counts2[:-1], out=starts2[1:])
    pos = np.arange(len(src_s)) - starts2[cell_s]

    ag_s = cell_s % NGROUP
    blk_s = cell_s // NGROUP
    i16 = (src_s - ag_s * GROWS).astype(np.int16)

    # padded per-group index arrays [nblocks, cap_rows]; -1 tail padding
    # (skipped by dma_gather via the runtime count register)
    cnt = counts2.reshape(nblocks, NGROUP).astype(np.int32)
    idxpads = []
    for g in range(NGROUP):
        m = ag_s == g
        a = np.full((nblocks, caps[g]), -1, np.int16)
        a[blk_s[m], pos[m]] = i16[m]
        # >= 1 valid index per cell (dummy idx 0, weight 0)
        empty = cnt[:, g] == 0
        a[empty, 0] = 0
        idxpads.append(a)
    cnt = np.maximum(cnt, 1)
    cnt = np.ascontiguousarray(
        cnt.reshape(NCORES, 1, NB_PER_CORE * NGROUP))

    # slot/w columns [nblocks, P, TB1]
    slot = np.zeros((nblocks, P, TB1), np.float16)
    warr = np.zeros((nblocks, P, TB1), np.float16)
    tcol = np.take(TBASE, ag_s) + pos // P
    prow = pos % P
    slot[blk_s, prow, tcol] = (dst_s % P).astype(np.float16)
    warr[blk_s, prow, tcol] = w_s.astype(np.float16)
    slot[:, :, TB] = np.arange(P, dtype=np.float16)[None, :]
    warr[:, :, TB] = wself.astype(np.float16).reshape(nblocks, P)

    # wrap indices: per (core, call, group): [KBLK*cap] -> [128, KBLK*cap/16]
    ixparts = []
    for g in range(NGROUP):
        a = idxpads[g].reshape(NCORES, NCALL, KBLK * caps[g])
        a = a.reshape(NCORES, NCALL, KBLK * caps[g] // 16, 16)
        a = a.transpose(0, 1, 3, 2)                   # [8, 14, 16, cols]
        a = np.tile(a, (1, 1, 8, 1))                  # [8, 14, 128, cols]
        ixparts.append(a)
    ixd = np.ascontiguousarray(np.concatenate(ixparts, axis=3))

    sw = np.concatenate([slot, warr], axis=2)         # [nblocks, P, 2*TB1]
    sw = sw.reshape(NCORES, NCALL, KBLK, P, 2 * TB1)
    swd = np.ascontiguousarray(sw.transpose(0, 1, 3, 2, 4).reshape(
        NCORES, NCALL, P, KBLK * 2 * TB1))
    return ixd, swd, cnt


def run_layer(nc, z_f16, ixd, swd, cnt, W, b, tb1, *, trace=False,
              tmpdir=None):
    iot = np.tile(np.arange(P, dtype=np.float16), (P, tb1)) \
        .reshape(P, tb1 * P)
    in_maps = []
    for c in range(NCORES):
        in_maps.append({
            "zt": z_f16,
            "zself": z_f16[c * SHARD:(c + 1) * SHARD],
            "ixd": ixd[c],
            "swd": swd[c],
            "cnt": cnt[c],
            "iot": iot,
            "wt": np.ascontiguousarray(W.astype(np.float16)),
            "brow": np.ascontiguousarray(
                b.astype(np.float16).reshape(1, D)),
        })
    res = bass_utils.run_bass_kernel_spmd(
        nc, in_maps, core_ids=list(range(NCORES)), trace=trace, tmpdir=tmpdir,
    )
    out = np.concatenate([res.results[c]["out"] for c in range(NCORES)],
                         axis=0)
    return out, res


def _enable_tracing():
    """Install the NTFF profile hook that this image's antenv lacks, and
    neuter the artifact upload (no bucket access here)."""
    import sys
    import types
    try:
        import antenv.axon_hooks  # noqa: F401
        have = True
    except ImportError:
        have = False
    if not have:
        mod = types.ModuleType("antenv.axon_hooks")
        mod._hook = None

        def set_axon_ntff_profile_hook(h):
            mod._hook = h

        def get_axon_ntff_profile_hook():
            return mod._hook

        mod.set_axon_ntff_profile_hook = set_axon_ntff_profile_hook
        mod.get_axon_ntff_profile_hook = get_axon_ntff_profile_hook
        sys.modules["antenv.axon_hooks"] = mod
        from trn_agent_boot.trn_boot import _ntff_profile_via_ctypes
        hook = _ntff_profile_via_ctypes("/opt/axon/libaxon_pjrt.so")
        mod.set_axon_ntff_profile_hook(hook)
    bass_utils.upload_artifacts = lambda tmpdir: f"local:{tmpdir}"


def kernel(x, edge_index, edge_weight, W1, b1, W2, b2):
    x = np.asarray(x, dtype=np.float32)
    edge_index = np.asarray(edge_index)
    edge_weight = np.asarray(edge_weight, dtype=np.float32)
    src = edge_index[0].astype(np.int64)
    dst = edge_index[1].astype(np.int64)

    try:
        capst = CAPS
        ixd, swd, cnt = preprocess(src, dst, edge_weight, capst)
    except RuntimeError:
        capst = CAPS_FALLBACK
        ixd, swd, cnt = preprocess(src, dst, edge_weight, capst)
    tb1 = sum(capst) + 1

    key = (capst, PREP, SBUILD_TILE)
    if key not in _nc_cache:
        _nc_cache[key] = build_nc(capst, PREP)
    nc = _nc_cache[key]

    trace = bool(int(os.environ.get("GCN_TRACE", "0")))
    if trace:
        _enable_tracing()

    z1 = np.zeros((ZROWS, D), np.float16)
    z1[:N_NODES] = x.astype(np.float16)
    h1, res1 = run_layer(nc, z1, ixd, swd, cnt, W1, b1, tb1, trace=trace)

    z2 = np.zeros((ZROWS, D), np.float16)
    z2[:N_PAD] = h1.astype(np.float16)
    h2, res2 = run_layer(nc, z2, ixd, swd, cnt, W2, b2, tb1, trace=trace)

    if trace:
        t1 = res1.exec_time_ns or 0
        t2 = res2.exec_time_ns or 0
        print(f"[kernel] layer1 exec: {t1} ns, layer2 exec: {t2} ns, "
              f"total: {t1 + t2} ns")
        kernel.last_exec_ns = t1 + t2
        kernel.last_results = (res1, res2)

    return h2[:N_NODES].astype(np.float32)


# revision 17
# speedup vs baseline: 2.5240x; 2.5240x over previous
"""Trainium2 Bass kernel for a 2-layer GCN (PyG GCNConv semantics).

Strategy (8 NeuronCores, SPMD, full I/O):
  - Host: fold symmetric deg^-1/2 normalization + edge weight into one
    per-edge scalar w~ = dinv[src]*w*dinv[dst]. Self-loops skip the gather
    entirely: each core's own contiguous block rows are loaded sequentially
    and folded in via a trailing host-built diagonal S tile (w~ = dinv^2).
    Sort edges by (dst block, src group). Destinations are
    partitioned contiguously across 8 cores (12544 padded nodes each =
    98 blocks of 128). Sources are split into 4 groups of 25088 rows so
    int16 indices work with the fast dma_gather path (4 parallel SWDGE
    queues). The one-hot scatter matrices S (graph-only, shared by both
    layers) are precomputed on the host and streamed from DRAM.
  - Device, per layer (aggregate-first: out = relu((A_hat z) W + b)),
    per dst block:
      for g in 0..3 (parallel SWDGE queues):
        G_g = dma_gather(z_group_g, idx16)      [128e, TBG*128] fp16
              (-1 indices at each group tail are skipped; the runtime
               count comes from a reg_load of the counts table)
      PSUM aggT[f, n] += G_t.T @ S_t  over tiles (TensorE, fp32 accum)
      out[n, :] = relu(aggT.T @ W + ones.T @ b)  (TensorE f32 + ScalarE)
  - Two launches (one per GCN layer) of the same compiled program; host
    concatenates layer-1 shards, casts to fp16, feeds layer 2.

fp16 data path gives ~2e-4 relative error vs the f32 reference.
"""

import os
from contextlib import ExitStack

import numpy as np

import concourse.bacc as bacc
import concourse.bass as bass
import concourse.mybir as mybir
import concourse.tile as tile
from concourse.tile import add_dep_helper
from concourse import bass_utils

P = 128          # partitions / block size / feature dim
D = 128
NCORES = 8
NGROUP = 4                  # src groups (int16 index range)
N_NODES = 100000
NB_PER_CORE = 98            # blocks of 128 dst nodes per core
SHARD = NB_PER_CORE * P     # 12544
N_PAD = SHARD * NCORES      # 100352
GBUFS = 8                   # G pool depth (memset-guarded for -1 skips)

_nc_cache = {}


def build_nc(nb, tbg, nt_rows):
    """Per-core SPMD program: one GCN layer (aggregate + transform)."""
    dt = mybir.dt
    grows = nt_rows // NGROUP
    tb = NGROUP * tbg                 # total tiles per block
    six = tb * 8                      # idx cols (int16): NGROUP * tbg*128/16
    nc = bacc.Bacc(
        "TRN2",
        target_bir_lowering=False,
        debug=False,
        enable_asserts=False,
        num_devices=1,
        num_swdge_queues=4,
    )
    zt = nc.dram_tensor("zt", [nt_rows, D], dt.float16, kind="ExternalInput")
    ixd = nc.dram_tensor("ixd", [nb, P, six], dt.int16, kind="ExternalInput")
    swd = nc.dram_tensor("swd", [nb, P, (tb + 1) * P], dt.float16,
                         kind="ExternalInput")
    zself = nc.dram_tensor("zself", [nb * P, D], dt.float16,
                           kind="ExternalInput")
    cnt = nc.dram_tensor("cnt", [1, nb * NGROUP], dt.int32, kind="ExternalInput")
    wt = nc.dram_tensor("wt", [D, D], dt.float32, kind="ExternalInput")
    brow = nc.dram_tensor("brow", [1, D], dt.float32, kind="ExternalInput")
    out = nc.dram_tensor("out", [nb * P, D], dt.float32, kind="ExternalOutput")

    with tile.TileContext(nc) as tc, ExitStack() as ctx:
        const = ctx.enter_context(tc.tile_pool(name="const", bufs=1))
        meta = ctx.enter_context(tc.tile_pool(name="meta", bufs=6))
        gpools = [
            ctx.enter_context(tc.tile_pool(name=f"g{g}", bufs=GBUFS))
            for g in range(NGROUP)
        ]
        spool = ctx.enter_context(tc.tile_pool(name="s", bufs=6))
        apool = ctx.enter_context(tc.tile_pool(name="agg", bufs=4))
        opool = ctx.enter_context(tc.tile_pool(name="o", bufs=4))
        ppool = ctx.enter_context(tc.tile_pool(name="ps", bufs=4, space="PSUM"))
        p2pool = ctx.enter_context(tc.tile_pool(name="ps2", bufs=2, space="PSUM"))

        w_t = const.tile([D, D], dt.float32)
        nc.sync.dma_start(out=w_t[:], in_=wt[:])
        b_t = const.tile([1, D], dt.float32)
        nc.sync.dma_start(out=b_t[:], in_=brow[:])
        ones_t = const.tile([1, P], dt.float32)
        nc.vector.memset(ones_t[:], 1.0)
        cnt_t = const.tile([1, nb * NGROUP], dt.int32)
        nc.sync.dma_start(out=cnt_t[:], in_=cnt[:])

        cap16 = tbg * 8                 # idx cols per group
        prev_gather = None
        for b in range(nb):
            ix = meta.tile([P, six], dt.int16, tag="ix")
            nc.sync.dma_start(out=ix[:], in_=ixd[b])
            s_w = spool.tile([P, (tb + 1) * P], dt.float16, tag="S")
            nc.scalar.dma_start(out=s_w[:], in_=swd[b])
            zs = opool.tile([P, D], dt.float16, tag="zs")
            nc.sync.dma_start(out=zs[:], in_=zself[b * P:(b + 1) * P, :])

            regs = [nc.gpsimd.alloc_register(f"cnt_{b}_{g}")
                    for g in range(NGROUP)]
            ld = nc.gpsimd.reg_load(
                regs, cnt_t[0:1, b * NGROUP:(b + 1) * NGROUP])
            if prev_gather is not None:
                # keep count registers' live ranges short: don't let the
                # scheduler hoist loads far ahead of their gathers
                add_dep_helper(ld.ins, prev_gather.ins, sync=False,
                               reason="limit cnt register liveness")
            g_tiles = []
            for g in range(NGROUP):
                g_w = gpools[g].tile([P, tbg * P], dt.float16, tag=f"G{g}")
                if b < GBUFS:
                    # first pass over each pool buffer: clear stale SBUF so
                    # rows skipped by -1 indices can't be NaN (w~=0 * NaN)
                    nc.vector.memset(g_w[:], 0.0)
                prev_gather = nc.gpsimd.dma_gather(
                    out_ap=g_w[:].rearrange("p (j n) -> p j n", n=P),
                    in_ap=zt[g * grows:(g + 1) * grows, :],
                    idxs_ap=ix[:, g * cap16:(g + 1) * cap16],
                    num_idxs=tbg * P,
                    num_idxs_reg=regs[g],
                    elem_size=P,
                    queue_num=g,
                    single_packet=False,
                )
                g_tiles.extend(g_w[:, j * P:(j + 1) * P] for j in range(tbg))

            psum = ppool.tile([P, P], dt.float32, tag="psA")
            for t in range(tb):
                nc.tensor.matmul(
                    out=psum[:],
                    lhsT=g_tiles[t],
                    rhs=s_w[:, t * P:(t + 1) * P],
                    start=(t == 0),
                    stop=False,
                )
            # self-loop contribution: plain sequential load, diagonal S tile
            nc.tensor.matmul(out=psum[:], lhsT=zs[:],
                             rhs=s_w[:, tb * P:(tb + 1) * P],
                             start=False, stop=True)

            agg_t = apool.tile([P, P], dt.float32, tag="aggT")
            nc.scalar.activation(out=agg_t[:], in_=psum[:],
                                 func=mybir.ActivationFunctionType.Copy)

            psum2 = p2pool.tile([P, D], dt.float32, tag="psB")
            nc.tensor.matmul(out=psum2[:], lhsT=agg_t[:], rhs=w_t[:],
                             start=True, stop=False)
            nc.tensor.matmul(out=psum2[:], lhsT=ones_t[:], rhs=b_t[:],
                             start=False, stop=True)

            o_t = opool.tile([P, D], dt.float32, tag="o")
            nc.scalar.activation(out=o_t[:], in_=psum2[:],
                                 func=mybir.ActivationFunctionType.Relu)
            nc.sync.dma_start(out=out[b * P:(b + 1) * P, :], in_=o_t[:])

    nc.compile()
    return nc


def preprocess(src, dst, ew, n_nodes, ncores, nb_per_core):
    """Per-core metadata for the dma_gather kernel.

    Returns (ixd, swd, cnt, tbg):
      ixd: [ncores, nb, P, NGROUP*tbg*8] int16 wrapped gather indices,
           replicated across the 8 q7 stripes; -1 padding at group tails
      swd: [ncores, nb, P, NGROUP*tbg*P] fp16 host-built scatter matrices
      cnt: [ncores, 1, nb*NGROUP] int32 real index count per (block, group)
    """
    shard = nb_per_core * P
    n_pad = shard * ncores
    grows = n_pad // NGROUP
    deg = np.bincount(dst, weights=ew.astype(np.float64), minlength=n_nodes) + 1.0
    dinv = (1.0 / np.sqrt(deg)).astype(np.float32)
    s_all = src
    d_all = dst
    wtil = dinv[s_all] * ew.astype(np.float32) * dinv[d_all]
    wself = np.zeros(n_pad, np.float32)
    wself[:n_nodes] = dinv * dinv            # self-loop weight 1 * dinv^2

    blk = d_all // P
    grp = s_all // grows
    cell = blk * NGROUP + grp
    order = np.lexsort((s_all, cell))
    s_s = s_all[order]
    d_s = d_all[order]
    w_s = wtil[order]
    cell_s = cell[order]

    nblocks = ncores * nb_per_core
    ncells = nblocks * NGROUP
    counts = np.bincount(cell_s, minlength=ncells)
    tbg = max(1, int(-(-counts.max() // P)))
    cap = tbg * P
    starts = np.zeros(ncells, np.int64)
    np.cumsum(counts[:-1], out=starts[1:])
    pos = np.arange(len(d_s)) - starts[cell_s]

    idxp = np.full((ncells, cap), -1, np.int16)
    wp = np.zeros((ncells, cap), np.float16)
    slotp = np.zeros((ncells, cap), np.int16)
    flat = cell_s * cap + pos
    idxp.reshape(-1)[flat] = (s_s % grows).astype(np.int16)
    wp.reshape(-1)[flat] = w_s
    slotp.reshape(-1)[flat] = (d_s % P).astype(np.int16)
    # >= 1 valid index per cell (empty cells get a dummy idx 0 with w~ = 0)
    empty = counts == 0
    idxp[empty, 0] = 0
    cnt = np.maximum(counts, 1).astype(np.int32)

    # idx: [ncells, cap] -> wrapped [ncells, 16, cap/16] -> 8x stripes
    ixw = idxp.reshape(ncells, cap // 16, 16).transpose(0, 2, 1)
    ixw = np.tile(ixw, (1, 8, 1))
    ixd = ixw.reshape(ncores, nb_per_core, NGROUP, P, cap // 16)
    ixd = np.ascontiguousarray(ixd.transpose(0, 1, 3, 2, 4)).reshape(
        ncores, nb_per_core, P, NGROUP * cap // 16)

    # host-built scatter matrices: S[cell, j, p, n] = w~ * (slot == n)
    onehot = (slotp[:, :, None] == np.arange(P, dtype=np.int16)[None, None, :])
    sw = onehot.astype(np.float16) * wp[:, :, None]       # [ncells, cap, P]
    sw = sw.reshape(ncores, nb_per_core, NGROUP, tbg, P, P)
    sw = np.ascontiguousarray(sw.transpose(0, 1, 4, 2, 3, 5)).reshape(
        ncores, nb_per_core, P, NGROUP * tbg * P)
    # trailing diagonal tile: self-loop contribution (no gather needed)
    diag = (np.eye(P, dtype=np.float16)[None, None] *
            wself.astype(np.float16).reshape(ncores, nb_per_core, P)[..., None, :])
    swd = np.concatenate([sw, diag.reshape(ncores, nb_per_core, P, P)], axis=3)

    cnt = np.ascontiguousarray(cnt.reshape(ncores, 1, nb_per_core * NGROUP))
    return ixd, swd, cnt, tbg


def run_layer(nc, z_f16, ixd, swd, cnt, W, b, *, trace=False, tmpdir=None):
    ncores = ixd.shape[0]
    shard = ixd.shape[1] * P
    in_maps = []
    for c in range(ncores):
        in_maps.append({
            "zt": z_f16,
            "zself": z_f16[c * shard:(c + 1) * shard],
            "ixd": ixd[c],
            "swd": swd[c],
            "cnt": cnt[c],
            "wt": np.ascontiguousarray(W.astype(np.float32)),
            "brow": np.ascontiguousarray(b.astype(np.float32).reshape(1, D)),
        })
    res = bass_utils.run_bass_kernel_spmd(
        nc, in_maps, core_ids=list(range(ncores)), trace=trace, tmpdir=tmpdir,
    )
    out = np.concatenate([res.results[c]["out"] for c in range(ncores)], axis=0)
    return out, res


def _enable_tracing():
    """Install the NTFF profile hook that this image's antenv lacks, and
    neuter the artifact upload (no bucket access here)."""
    import sys
    import types
    try:
        import antenv.axon_hooks  # noqa: F401
        have = True
    except ImportError:
        have = False
    if not have:
        mod = types.ModuleType("antenv.axon_hooks")
        mod._hook = None

        def set_axon_ntff_profile_hook(h):
            mod._hook = h

        def get_axon_ntff_profile_hook():
            return mod._hook

        mod.set_axon_ntff_profile_hook = set_axon_ntff_profile_hook
        mod.get_axon_ntff_profile_hook = get_axon_ntff_profile_hook
        sys.modules["antenv.axon_hooks"] = mod
        from trn_agent_boot.trn_boot import _ntff_profile_via_ctypes
        hook = _ntff_profile_via_ctypes("/opt/axon/libaxon_pjrt.so")
        mod.set_axon_ntff_profile_hook(hook)
    bass_utils.upload_artifacts = lambda tmpdir: f"local:{tmpdir}"


def kernel(x, edge_index, edge_weight, W1, b1, W2, b2):
    x = np.asarray(x, dtype=np.float32)
    edge_index = np.asarray(edge_index)
    edge_weight = np.asarray(edge_weight, dtype=np.float32)
    src = edge_index[0].astype(np.int64)
    dst = edge_index[1].astype(np.int64)

    ixd, swd, cnt, tbg = preprocess(src, dst, edge_weight,
                                    N_NODES, NCORES, NB_PER_CORE)

    key = (NB_PER_CORE, tbg, N_PAD)
    if key not in _nc_cache:
        _nc_cache[key] = build_nc(NB_PER_CORE, tbg, N_PAD)
    nc = _nc_cache[key]

    trace = bool(int(os.environ.get("GCN_TRACE", "0")))
    if trace:
        _enable_tracing()

    z1 = np.zeros((N_PAD, D), np.float16)
    z1[:N_NODES] = x.astype(np.float16)
    h1, res1 = run_layer(nc, z1, ixd, swd, cnt, W1, b1, trace=trace)

    z2 = h1.astype(np.float16)
    h2, res2 = run_layer(nc, z2, ixd, swd, cnt, W2, b2, trace=trace)

    if trace:
        t1 = res1.exec_time_ns or 0
        t2 = res2.exec_time_ns or 0
        print(f"[kernel] layer1 exec: {t1} ns, layer2 exec: {t2} ns, "
              f"total: {t1 + t2} ns")
        kernel.last_exec_ns = t1 + t2
        kernel.last_results = (res1, res2)

    return h2[:N_NODES].astype(np.float32)



# revision 19
# speedup vs baseline: 2.5579x; 1.0134x over previous
"""Trainium2 Bass kernel for a 2-layer GCN (PyG GCNConv semantics).

Strategy (8 NeuronCores, SPMD, full I/O):
  - Host: fold symmetric deg^-1/2 normalization + edge weight into one
    per-edge scalar w~ = dinv[src]*w*dinv[dst]. Self-loops skip the gather
    entirely: each core's own contiguous block rows are loaded sequentially
    and folded in via a trailing host-built diagonal S tile (w~ = dinv^2).
    Sort edges by (dst block, src group). Destinations are
    partitioned contiguously across 8 cores (12544 padded nodes each =
    98 blocks of 128). Sources are split into 4 groups of 25088 rows so
    int16 indices work with the fast dma_gather path (4 parallel SWDGE
    queues). The one-hot scatter matrices S (graph-only, shared by both
    layers) are precomputed on the host and streamed from DRAM.
  - Device, per layer (aggregate-first: out = relu((A_hat z) W + b)),
    per dst block:
      for g in 0..3 (parallel SWDGE queues):
        G_g = dma_gather(z_group_g, idx16)      [128e, TBG*128] fp16
              (-1 indices at each group tail are skipped; the runtime
               count comes from a reg_load of the counts table)
      PSUM aggT[f, n] += G_t.T @ S_t  over tiles (TensorE, fp32 accum)
      out[n, :] = relu(aggT.T @ W + ones.T @ b)  (TensorE f32 + ScalarE)
  - Two launches (one per GCN layer) of the same compiled program; host
    concatenates layer-1 shards, casts to fp16, feeds layer 2.

fp16 data path gives ~2e-4 relative error vs the f32 reference.
"""

import os

# Defensive: a previous process can leave /dev/neuron* in a stale state that
# silently corrupts results (observed once in testing); a core reset at
# runtime open costs wall-clock only, not measured HW exec time.
os.environ.setdefault("NEURON_RT_RESET_CORES", "1")

from contextlib import ExitStack

import numpy as np

import concourse.bacc as bacc
import concourse.bass as bass
import concourse.mybir as mybir
import concourse.tile as tile
from concourse.tile import add_dep_helper
from concourse import bass_utils

P = 128          # partitions / block size / feature dim
D = 128
NCORES = 8
NGROUP = 4                  # src groups (int16 index range)
N_NODES = 100000
NB_PER_CORE = 98            # blocks of 128 dst nodes per core
SHARD = NB_PER_CORE * P     # 12544
N_PAD = SHARD * NCORES      # 100352
GBUFS = 4                   # G pool depth (memset-guarded for -1 skips)

_nc_cache = {}


def build_nc(nb, tbg, nt_rows):
    """Per-core SPMD program: one GCN layer (aggregate + transform)."""
    dt = mybir.dt
    grows = nt_rows // NGROUP
    tb = NGROUP * tbg                 # total tiles per block
    six = tb * 8                      # idx cols (int16): NGROUP * tbg*128/16
    nc = bacc.Bacc(
        "TRN2",
        target_bir_lowering=False,
        debug=False,
        enable_asserts=False,
        num_devices=1,
        num_swdge_queues=4,
    )
    zt = nc.dram_tensor("zt", [nt_rows, D], dt.float16, kind="ExternalInput")
    ixd = nc.dram_tensor("ixd", [nb, P, six], dt.int16, kind="ExternalInput")
    swd = nc.dram_tensor("swd", [nb, P, (tb + 1) * P], dt.float16,
                         kind="ExternalInput")
    zself = nc.dram_tensor("zself", [nb * P, D], dt.float16,
                           kind="ExternalInput")
    cnt = nc.dram_tensor("cnt", [1, nb * NGROUP], dt.int32, kind="ExternalInput")
    wt = nc.dram_tensor("wt", [D, D], dt.float32, kind="ExternalInput")
    brow = nc.dram_tensor("brow", [1, D], dt.float32, kind="ExternalInput")
    out = nc.dram_tensor("out", [nb * P, D], dt.float32, kind="ExternalOutput")

    with tile.TileContext(nc) as tc, ExitStack() as ctx:
        const = ctx.enter_context(tc.tile_pool(name="const", bufs=1))
        meta = ctx.enter_context(tc.tile_pool(name="meta", bufs=4))
        gpools = [
            ctx.enter_context(tc.tile_pool(name=f"g{g}", bufs=GBUFS))
            for g in range(NGROUP)
        ]
        spool = ctx.enter_context(tc.tile_pool(name="s", bufs=4))
        apool = ctx.enter_context(tc.tile_pool(name="agg", bufs=3))
        opool = ctx.enter_context(tc.tile_pool(name="o", bufs=3))
        ppool = ctx.enter_context(tc.tile_pool(name="ps", bufs=2, space="PSUM"))
        p2pool = ctx.enter_context(tc.tile_pool(name="ps2", bufs=2, space="PSUM"))

        w_t = const.tile([D, D], dt.float32)
        nc.sync.dma_start(out=w_t[:], in_=wt[:])
        b_t = const.tile([1, D], dt.float32)
        nc.sync.dma_start(out=b_t[:], in_=brow[:])
        ones_t = const.tile([1, P], dt.float32)
        nc.vector.memset(ones_t[:], 1.0)
        cnt_t = const.tile([1, nb * NGROUP], dt.int32)
        nc.sync.dma_start(out=cnt_t[:], in_=cnt[:])

        cap16 = tbg * 8                 # idx cols per group
        prev_gather = None
        for b in range(nb):
            ix = meta.tile([P, six], dt.int16, tag="ix")
            nc.sync.dma_start(out=ix[:], in_=ixd[b])
            s_w = spool.tile([P, (tb + 1) * P], dt.float16, tag="S")
            nc.scalar.dma_start(out=s_w[:], in_=swd[b])
            zs = opool.tile([P, D], dt.float16, tag="zs")
            nc.sync.dma_start(out=zs[:], in_=zself[b * P:(b + 1) * P, :])

            regs = [nc.gpsimd.alloc_register(f"cnt_{b}_{g}")
                    for g in range(NGROUP)]
            ld = nc.gpsimd.reg_load(
                regs, cnt_t[0:1, b * NGROUP:(b + 1) * NGROUP])
            if prev_gather is not None:
                # keep count registers' live ranges short: don't let the
                # scheduler hoist loads far ahead of their gathers
                add_dep_helper(ld.ins, prev_gather.ins, sync=False,
                               reason="limit cnt register liveness")
            g_tiles = []
            for g in range(NGROUP):
                g_w = gpools[g].tile([P, tbg * P], dt.float16, tag=f"G{g}")
                if b < GBUFS:
                    # first pass over each pool buffer: clear stale SBUF so
                    # rows skipped by -1 indices can't be NaN (w~=0 * NaN)
                    nc.vector.memset(g_w[:], 0.0)
                prev_gather = nc.gpsimd.dma_gather(
                    out_ap=g_w[:].rearrange("p (j n) -> p j n", n=P),
                    in_ap=zt[g * grows:(g + 1) * grows, :],
                    idxs_ap=ix[:, g * cap16:(g + 1) * cap16],
                    num_idxs=tbg * P,
                    num_idxs_reg=regs[g],
                    elem_size=P,
                    queue_num=g,
                    single_packet=False,
                )
                g_tiles.extend(g_w[:, j * P:(j + 1) * P] for j in range(tbg))

            psum = ppool.tile([P, P], dt.float32, tag="psA")
            for t in range(tb):
                nc.tensor.matmul(
                    out=psum[:],
                    lhsT=g_tiles[t],
                    rhs=s_w[:, t * P:(t + 1) * P],
                    start=(t == 0),
                    stop=False,
                )
            # self-loop contribution: plain sequential load, diagonal S tile
            nc.tensor.matmul(out=psum[:], lhsT=zs[:],
                             rhs=s_w[:, tb * P:(tb + 1) * P],
                             start=False, stop=True)

            agg_t = apool.tile([P, P], dt.float32, tag="aggT")
            nc.scalar.activation(out=agg_t[:], in_=psum[:],
                                 func=mybir.ActivationFunctionType.Copy)

            psum2 = p2pool.tile([P, D], dt.float32, tag="psB")
            nc.tensor.matmul(out=psum2[:], lhsT=agg_t[:], rhs=w_t[:],
                             start=True, stop=False)
            nc.tensor.matmul(out=psum2[:], lhsT=ones_t[:], rhs=b_t[:],
                             start=False, stop=True)

            o_t = opool.tile([P, D], dt.float32, tag="o")
            nc.scalar.activation(out=o_t[:], in_=psum2[:],
                                 func=mybir.ActivationFunctionType.Relu)
            nc.sync.dma_start(out=out[b * P:(b + 1) * P, :], in_=o_t[:])

    nc.compile()
    return nc


def preprocess(src, dst, ew, n_nodes, ncores, nb_per_core):
    """Per-core metadata for the dma_gather kernel.

    Returns (ixd, swd, cnt, tbg):
      ixd: [ncores, nb, P, NGROUP*tbg*8] int16 wrapped gather indices,
           replicated across the 8 q7 stripes; -1 padding at group tails
      swd: [ncores, nb, P, NGROUP*tbg*P] fp16 host-built scatter matrices
      cnt: [ncores, 1, nb*NGROUP] int32 real index count per (block, group)
    """
    shard = nb_per_core * P
    n_pad = shard * ncores
    grows = n_pad // NGROUP
    deg = np.bincount(dst, weights=ew.astype(np.float64), minlength=n_nodes) + 1.0
    dinv = (1.0 / np.sqrt(deg)).astype(np.float32)
    s_all = src
    d_all = dst
    wtil = dinv[s_all] * ew.astype(np.float32) * dinv[d_all]
    wself = np.zeros(n_pad, np.float32)
    wself[:n_nodes] = dinv * dinv            # self-loop weight 1 * dinv^2

    blk = d_all // P
    grp = s_all // grows
    cell = blk * NGROUP + grp
    order = np.lexsort((s_all, cell))
    s_s = s_all[order]
    d_s = d_all[order]
    w_s = wtil[order]
    cell_s = cell[order]

    nblocks = ncores * nb_per_core
    ncells = nblocks * NGROUP
    counts = np.bincount(cell_s, minlength=ncells)
    tbg = max(1, int(-(-counts.max() // P)))
    cap = tbg * P
    starts = np.zeros(ncells, np.int64)
    np.cumsum(counts[:-1], out=starts[1:])
    pos = np.arange(len(d_s)) - starts[cell_s]

    idxp = np.full((ncells, cap), -1, np.int16)
    wp = np.zeros((ncells, cap), np.float16)
    slotp = np.zeros((ncells, cap), np.int16)
    flat = cell_s * cap + pos
    idxp.reshape(-1)[flat] = (s_s % grows).astype(np.int16)
    wp.reshape(-1)[flat] = w_s
    slotp.reshape(-1)[flat] = (d_s % P).astype(np.int16)
    # >= 1 valid index per cell (empty cells get a dummy idx 0 with w~ = 0)
    empty = counts == 0
    idxp[empty, 0] = 0
    cnt = np.maximum(counts, 1).astype(np.int32)

    # idx: [ncells, cap] -> wrapped [ncells, 16, cap/16] -> 8x stripes
    ixw = idxp.reshape(ncells, cap // 16, 16).transpose(0, 2, 1)
    ixw = np.tile(ixw, (1, 8, 1))
    ixd = ixw.reshape(ncores, nb_per_core, NGROUP, P, cap // 16)
    ixd = np.ascontiguousarray(ixd.transpose(0, 1, 3, 2, 4)).reshape(
        ncores, nb_per_core, P, NGROUP * cap // 16)

    # host-built scatter matrices: S[cell, j, p, n] = w~ * (slot == n)
    onehot = (slotp[:, :, None] == np.arange(P, dtype=np.int16)[None, None, :])
    sw = onehot.astype(np.float16) * wp[:, :, None]       # [ncells, cap, P]
    sw = sw.reshape(ncores, nb_per_core, NGROUP, tbg, P, P)
    sw = np.ascontiguousarray(sw.transpose(0, 1, 4, 2, 3, 5)).reshape(
        ncores, nb_per_core, P, NGROUP * tbg * P)
    # trailing diagonal tile: self-loop contribution (no gather needed)
    diag = (np.eye(P, dtype=np.float16)[None, None] *
            wself.astype(np.float16).reshape(ncores, nb_per_core, P)[..., None, :])
    swd = np.concatenate([sw, diag.reshape(ncores, nb_per_core, P, P)], axis=3)

    cnt = np.ascontiguousarray(cnt.reshape(ncores, 1, nb_per_core * NGROUP))
    return ixd, swd, cnt, tbg


def run_layer(nc, z_f16, ixd, swd, cnt, W, b, *, trace=False, tmpdir=None):
    ncores = ixd.shape[0]
    shard = ixd.shape[1] * P
    in_maps = []
    for c in range(ncores):
        in_maps.append({
            "zt": z_f16,
            "zself": z_f16[c * shard:(c + 1) * shard],
            "ixd": ixd[c],
            "swd": swd[c],
            "cnt": cnt[c],
            "wt": np.ascontiguousarray(W.astype(np.float32)),
            "brow": np.ascontiguousarray(b.astype(np.float32).reshape(1, D)),
        })
    res = bass_utils.run_bass_kernel_spmd(
        nc, in_maps, core_ids=list(range(ncores)), trace=trace, tmpdir=tmpdir,
    )
    out = np.concatenate([res.results[c]["out"] for c in range(ncores)], axis=0)
    return out, res


def _enable_tracing():
    """Install the NTFF profile hook that this image's antenv lacks, and
    neuter the artifact upload (no bucket access here)."""
    import sys
    import types
    try:
        import antenv.axon_hooks  # noqa: F401
        have = True
    except ImportError:
        have = False
    if not have:
        mod = types.ModuleType("antenv.axon_hooks")
        mod._hook = None

        def set_axon_ntff_profile_hook(h):
            mod._hook = h

        def get_axon_ntff_profile_hook():
            return mod._hook

        mod.set_axon_ntff_profile_hook = set_axon_ntff_profile_hook
        mod.get_axon_ntff_profile_hook = get_axon_ntff_profile_hook
        sys.modules["antenv.axon_hooks"] = mod
        from trn_agent_boot.trn_boot import _ntff_profile_via_ctypes
        hook = _ntff_profile_via_ctypes("/opt/axon/libaxon_pjrt.so")
        mod.set_axon_ntff_profile_hook(hook)
    bass_utils.upload_artifacts = lambda tmpdir: f"local:{tmpdir}"


def kernel(x, edge_index, edge_weight, W1, b1, W2, b2):
    x = np.asarray(x, dtype=np.float32)
    edge_index = np.asarray(edge_index)
    edge_weight = np.asarray(edge_weight, dtype=np.float32)
    src = edge_index[0].astype(np.int64)
    dst = edge_index[1].astype(np.int64)

    ixd, swd, cnt, tbg = preprocess(src, dst, edge_weight,
                                    N_NODES, NCORES, NB_PER_CORE)

    key = (NB_PER_CORE, tbg, N_PAD)
    if key not in _nc_cache:
        _nc_cache[key] = build_nc(NB_PER_CORE, tbg, N_PAD)
    nc = _nc_cache[key]

    trace = bool(int(os.environ.get("GCN_TRACE", "0")))
    if trace:
        _enable_tracing()

    z1 = np.zeros((N_PAD, D), np.float16)
    z1[:N_NODES] = x.astype(np.float16)
    h1, res1 = run_layer(nc, z1, ixd, swd, cnt, W1, b1, trace=trace)

    z2 = h1.astype(np.float16)
    h2, res2 = run_layer(nc, z2, ixd, swd, cnt, W2, b2, trace=trace)

    if trace:
        t1 = res1.exec_time_ns or 0
        t2 = res2.exec_time_ns or 0
        print(f"[kernel] layer1 exec: {t1} ns, layer2 exec: {t2} ns, "
              f"total: {t1 + t2} ns")
        kernel.last_exec_ns = t1 + t2
        kernel.last_results = (res1, res2)

    return h2[:N_NODES].astype(np.float32)



# revision 20
# speedup vs baseline: 3.0492x; 1.1920x over previous
"""Trainium2 Bass kernel for a 2-layer GCN (PyG GCNConv semantics).

Strategy (8 NeuronCores, SPMD, full I/O):
  - Host: fold symmetric deg^-1/2 normalization + edge weight into one
    per-edge scalar w~ = dinv[src]*w*dinv[dst]. Self-loops skip the gather
    entirely: each core's own contiguous block rows are loaded sequentially
    and folded in via a trailing host-built diagonal S tile (w~ = dinv^2).
    Sort edges by (dst block, src group). Destinations are
    partitioned contiguously across 8 cores (12544 padded nodes each =
    98 blocks of 128). Sources are split into 4 groups of 25088 rows so
    int16 indices work with the fast dma_gather path (4 parallel SWDGE
    queues). The one-hot scatter matrices S (graph-only, shared by both
    layers) are precomputed on the host and streamed from DRAM.
  - Device, per layer (aggregate-first: out = relu((A_hat z) W + b)),
    per dst block:
      for g in 0..3 (parallel SWDGE queues):
        G_g = dma_gather(z_group_g, idx16)      [128e, TBG*128] fp16
              (-1 indices at each group tail are skipped; the runtime
               count comes from a reg_load of the counts table)
      PSUM aggT[f, n] += G_t.T @ S_t  over tiles (TensorE, fp32 accum)
      out[n, :] = relu(aggT.T @ W + ones.T @ b)  (TensorE f32 + ScalarE)
  - Two launches (one per GCN layer) of the same compiled program; host
    concatenates layer-1 shards, casts to fp16, feeds layer 2.

fp16 data path gives ~2e-4 relative error vs the f32 reference.
"""

import os

# Defensive: a previous process can leave /dev/neuron* in a stale state that
# silently corrupts results (observed once in testing); a core reset at
# runtime open costs wall-clock only, not measured HW exec time.
os.environ.setdefault("NEURON_RT_RESET_CORES", "1")

from contextlib import ExitStack

import numpy as np

import concourse.bacc as bacc
import concourse.bass as bass
import concourse.mybir as mybir
import concourse.tile as tile
from concourse.tile import add_dep_helper
from concourse import bass_utils

P = 128          # partitions / block size / feature dim
D = 128
NCORES = 8
NGROUP = 4                  # src groups (int16 index range)
N_NODES = 100000
NB_PER_CORE = 98            # blocks of 128 dst nodes per core
SHARD = NB_PER_CORE * P     # 12544
N_PAD = SHARD * NCORES      # 100352
GBUFS = 4                   # G pool depth (memset-guarded for -1 skips)

_nc_cache = {}


def build_nc(nb, tbg, nt_rows):
    """Per-core SPMD program: one GCN layer (aggregate + transform)."""
    dt = mybir.dt
    grows = nt_rows // NGROUP
    tb = NGROUP * tbg                 # total tiles per block
    six = tb * 8                      # idx cols (int16): NGROUP * tbg*128/16
    nc = bacc.Bacc(
        "TRN2",
        target_bir_lowering=False,
        debug=False,
        enable_asserts=False,
        num_devices=1,
        num_swdge_queues=4,
    )
    zt = nc.dram_tensor("zt", [nt_rows, D], dt.float16, kind="ExternalInput")
    ixd = nc.dram_tensor("ixd", [nb, P, six], dt.int16, kind="ExternalInput")
    swd = nc.dram_tensor("swd", [nb, P, (tb + 1) * P], dt.float16,
                         kind="ExternalInput")
    zself = nc.dram_tensor("zself", [nb * P, D], dt.float16,
                           kind="ExternalInput")
    cnt = nc.dram_tensor("cnt", [1, nb * NGROUP], dt.int32, kind="ExternalInput")
    wt = nc.dram_tensor("wt", [D, D], dt.float32, kind="ExternalInput")
    brow = nc.dram_tensor("brow", [1, D], dt.float32, kind="ExternalInput")
    out = nc.dram_tensor("out", [nb * P, D], dt.float32, kind="ExternalOutput")

    with tile.TileContext(nc) as tc, ExitStack() as ctx:
        const = ctx.enter_context(tc.tile_pool(name="const", bufs=1))
        meta = ctx.enter_context(tc.tile_pool(name="meta", bufs=4))
        gpools = [
            ctx.enter_context(tc.tile_pool(name=f"g{g}", bufs=GBUFS))
            for g in range(NGROUP)
        ]
        spool = ctx.enter_context(tc.tile_pool(name="s", bufs=4))
        apool = ctx.enter_context(tc.tile_pool(name="agg", bufs=3))
        opool = ctx.enter_context(tc.tile_pool(name="o", bufs=3))
        ppool = ctx.enter_context(tc.tile_pool(name="ps", bufs=2, space="PSUM"))
        p2pool = ctx.enter_context(tc.tile_pool(name="ps2", bufs=2, space="PSUM"))

        w_t = const.tile([D, D], dt.float32)
        nc.sync.dma_start(out=w_t[:], in_=wt[:])
        b_t = const.tile([1, D], dt.float32)
        nc.sync.dma_start(out=b_t[:], in_=brow[:])
        ones_t = const.tile([1, P], dt.float32)
        nc.vector.memset(ones_t[:], 1.0)
        cnt_t = const.tile([1, nb * NGROUP], dt.int32)
        nc.sync.dma_start(out=cnt_t[:], in_=cnt[:])

        cap16 = tbg * 8                 # idx cols per group
        prev_gather = None
        for b in range(nb):
            ix = meta.tile([P, six], dt.int16, tag="ix")
            nc.sync.dma_start(out=ix[:], in_=ixd[b])
            s_w = spool.tile([P, (tb + 1) * P], dt.float16, tag="S")
            nc.scalar.dma_start(out=s_w[:], in_=swd[b])
            zs = opool.tile([P, D], dt.float16, tag="zs")
            nc.sync.dma_start(out=zs[:], in_=zself[b * P:(b + 1) * P, :])

            regs = [nc.gpsimd.alloc_register(f"cnt_{b}_{g}")
                    for g in range(NGROUP)]
            ld = nc.gpsimd.reg_load(
                regs, cnt_t[0:1, b * NGROUP:(b + 1) * NGROUP])
            if prev_gather is not None:
                # keep count registers' live ranges short: don't let the
                # scheduler hoist loads far ahead of their gathers
                add_dep_helper(ld.ins, prev_gather.ins, sync=False,
                               reason="limit cnt register liveness")
            g_tiles = []
            for g in range(NGROUP):
                g_w = gpools[g].tile([P, tbg * P], dt.float16, tag=f"G{g}")
                if b < GBUFS:
                    # first pass over each pool buffer: clear stale SBUF so
                    # rows skipped by -1 indices can't be NaN (w~=0 * NaN)
                    nc.vector.memset(g_w[:], 0.0)
                prev_gather = nc.gpsimd.dma_gather(
                    out_ap=g_w[:].rearrange("p (j n) -> p j n", n=P),
                    in_ap=zt[g * grows:(g + 1) * grows, :],
                    idxs_ap=ix[:, g * cap16:(g + 1) * cap16],
                    num_idxs=tbg * P,
                    num_idxs_reg=regs[g],
                    elem_size=P,
                    queue_num=g,
                    single_packet=False,
                )
                g_tiles.extend(g_w[:, j * P:(j + 1) * P] for j in range(tbg))

            psum = ppool.tile([P, P], dt.float32, tag="psA")
            for t in range(tb):
                nc.tensor.matmul(
                    out=psum[:],
                    lhsT=g_tiles[t],
                    rhs=s_w[:, t * P:(t + 1) * P],
                    start=(t == 0),
                    stop=False,
                )
            # self-loop contribution: plain sequential load, diagonal S tile
            nc.tensor.matmul(out=psum[:], lhsT=zs[:],
                             rhs=s_w[:, tb * P:(tb + 1) * P],
                             start=False, stop=True)

            agg_t = apool.tile([P, P], dt.float32, tag="aggT")
            nc.scalar.activation(out=agg_t[:], in_=psum[:],
                                 func=mybir.ActivationFunctionType.Copy)

            psum2 = p2pool.tile([P, D], dt.float32, tag="psB")
            nc.tensor.matmul(out=psum2[:], lhsT=agg_t[:], rhs=w_t[:],
                             start=True, stop=False)
            nc.tensor.matmul(out=psum2[:], lhsT=ones_t[:], rhs=b_t[:],
                             start=False, stop=True)

            o_t = opool.tile([P, D], dt.float32, tag="o")
            nc.scalar.activation(out=o_t[:], in_=psum2[:],
                                 func=mybir.ActivationFunctionType.Relu)
            nc.sync.dma_start(out=out[b * P:(b + 1) * P, :], in_=o_t[:])

    nc.compile()
    return nc


def preprocess(src, dst, ew, n_nodes, ncores, nb_per_core):
    """Per-core metadata for the dma_gather kernel.

    Returns (ixd, swd, cnt, tbg):
      ixd: [ncores, nb, P, NGROUP*tbg*8] int16 wrapped gather indices,
           replicated across the 8 q7 stripes; -1 padding at group tails
      swd: [ncores, nb, P, NGROUP*tbg*P] fp16 host-built scatter matrices
      cnt: [ncores, 1, nb*NGROUP] int32 real index count per (block, group)
    """
    shard = nb_per_core * P
    n_pad = shard * ncores
    grows = n_pad // NGROUP
    deg = np.bincount(dst, weights=ew.astype(np.float64), minlength=n_nodes) + 1.0
    dinv = (1.0 / np.sqrt(deg)).astype(np.float32)
    s_all = src
    d_all = dst
    wtil = dinv[s_all] * ew.astype(np.float32) * dinv[d_all]
    wself = np.zeros(n_pad, np.float32)
    wself[:n_nodes] = dinv * dinv            # self-loop weight 1 * dinv^2

    blk = d_all // P
    grp = s_all // grows
    cell = blk * NGROUP + grp
    order = np.lexsort((s_all, cell))
    s_s = s_all[order]
    d_s = d_all[order]
    w_s = wtil[order]
    cell_s = cell[order]

    nblocks = ncores * nb_per_core
    ncells = nblocks * NGROUP
    counts = np.bincount(cell_s, minlength=ncells)
    tbg = max(1, int(-(-counts.max() // P)))
    cap = tbg * P
    starts = np.zeros(ncells, np.int64)
    np.cumsum(counts[:-1], out=starts[1:])
    pos = np.arange(len(d_s)) - starts[cell_s]

    idxp = np.full((ncells, cap), -1, np.int16)
    wp = np.zeros((ncells, cap), np.float16)
    slotp = np.zeros((ncells, cap), np.int16)
    flat = cell_s * cap + pos
    idxp.reshape(-1)[flat] = (s_s % grows).astype(np.int16)
    wp.reshape(-1)[flat] = w_s
    slotp.reshape(-1)[flat] = (d_s % P).astype(np.int16)
    # >= 1 valid index per cell (empty cells get a dummy idx 0 with w~ = 0)
    empty = counts == 0
    idxp[empty, 0] = 0
    cnt = np.maximum(counts, 1).astype(np.int32)

    # idx: [ncells, cap] -> wrapped [ncells, 16, cap/16] -> 8x stripes
    ixw = idxp.reshape(ncells, cap // 16, 16).transpose(0, 2, 1)
    ixw = np.tile(ixw, (1, 8, 1))
    ixd = ixw.reshape(ncores, nb_per_core, NGROUP, P, cap // 16)
    ixd = np.ascontiguousarray(ixd.transpose(0, 1, 3, 2, 4)).reshape(
        ncores, nb_per_core, P, NGROUP * cap // 16)

    # host-built scatter matrices: S[cell, j, p, n] = w~ * (slot == n)
    onehot = (slotp[:, :, None] == np.arange(P, dtype=np.int16)[None, None, :])
    sw = onehot.astype(np.float16) * wp[:, :, None]       # [ncells, cap, P]
    sw = sw.reshape(ncores, nb_per_core, NGROUP, tbg, P, P)
    sw = np.ascontiguousarray(sw.transpose(0, 1, 4, 2, 3, 5)).reshape(
        ncores, nb_per_core, P, NGROUP * tbg * P)
    # trailing diagonal tile: self-loop contribution (no gather needed)
    diag = (np.eye(P, dtype=np.float16)[None, None] *
            wself.astype(np.float16).reshape(ncores, nb_per_core, P)[..., None, :])
    swd = np.concatenate([sw, diag.reshape(ncores, nb_per_core, P, P)], axis=3)

    cnt = np.ascontiguousarray(cnt.reshape(ncores, 1, nb_per_core * NGROUP))
    return ixd, swd, cnt, tbg


def run_layer(nc, z_f16, ixd, swd, cnt, W, b, *, trace=False, tmpdir=None):
    ncores = ixd.shape[0]
    shard = ixd.shape[1] * P
    in_maps = []
    for c in range(ncores):
        in_maps.append({
            "zt": z_f16,
            "zself": z_f16[c * shard:(c + 1) * shard],
            "ixd": ixd[c],
            "swd": swd[c],
            "cnt": cnt[c],
            "wt": np.ascontiguousarray(W.astype(np.float32)),
            "brow": np.ascontiguousarray(b.astype(np.float32).reshape(1, D)),
        })
    res = bass_utils.run_bass_kernel_spmd(
        nc, in_maps, core_ids=list(range(ncores)), trace=trace, tmpdir=tmpdir,
    )
    out = np.concatenate([res.results[c]["out"] for c in range(ncores)], axis=0)
    return out, res


def _enable_tracing():
    """Install the NTFF profile hook that this image's antenv lacks, and
    neuter the artifact upload (no bucket access here)."""
    import sys
    import types
    try:
        import antenv.axon_hooks  # noqa: F401
        have = True
    except ImportError:
        have = False
    if not have:
        mod = types.ModuleType("antenv.axon_hooks")
        mod._hook = None

        def set_axon_ntff_profile_hook(h):
            mod._hook = h

        def get_axon_ntff_profile_hook():
            return mod._hook

        mod.set_axon_ntff_profile_hook = set_axon_ntff_profile_hook
        mod.get_axon_ntff_profile_hook = get_axon_ntff_profile_hook
        sys.modules["antenv.axon_hooks"] = mod
        from trn_agent_boot.trn_boot import _ntff_profile_via_ctypes
        hook = _ntff_profile_via_ctypes("/opt/axon/libaxon_pjrt.so")
        mod.set_axon_ntff_profile_hook(hook)
    bass_utils.upload_artifacts = lambda tmpdir: f"local:{tmpdir}"


def _spot_check(h_out, z_f16, W, b, src, dst, wtil, wself, nodes):
    """Host-side verification of one launch on a few dst nodes.

    The device has produced silently-corrupted results when /dev/neuron*
    was left in a stale state by a previous process; this detects that
    (observed corruption: ~0.26 relative error vs the ~2e-4 of the fp16
    data path) so the caller can reset and retry the launch.
    """
    m = np.isin(dst, nodes)
    s_m, d_m, w_m = src[m], dst[m], wtil[m]
    zf = z_f16.astype(np.float32)
    exp = np.zeros((len(nodes), D), np.float32)
    got = np.zeros((len(nodes), D), np.float32)
    for i, n in enumerate(nodes):
        e = d_m == n
        agg = w_m[e] @ zf[s_m[e]] if e.any() else 0.0
        agg = agg + wself[n] * zf[n]
        exp[i] = np.maximum(agg @ W + b, 0.0)
        got[i] = h_out[n]
    denom = np.linalg.norm(exp) + 1e-6
    return np.linalg.norm(got - exp) / denom < 0.02


def kernel(x, edge_index, edge_weight, W1, b1, W2, b2):
    x = np.asarray(x, dtype=np.float32)
    edge_index = np.asarray(edge_index)
    edge_weight = np.asarray(edge_weight, dtype=np.float32)
    src = edge_index[0].astype(np.int64)
    dst = edge_index[1].astype(np.int64)

    ixd, swd, cnt, tbg = preprocess(src, dst, edge_weight,
                                    N_NODES, NCORES, NB_PER_CORE)

    key = (NB_PER_CORE, tbg, N_PAD)
    if key not in _nc_cache:
        _nc_cache[key] = build_nc(NB_PER_CORE, tbg, N_PAD)
    nc = _nc_cache[key]

    trace = bool(int(os.environ.get("GCN_TRACE", "0")))
    if trace:
        _enable_tracing()

    deg = np.bincount(dst, weights=edge_weight.astype(np.float64),
                      minlength=N_NODES) + 1.0
    dinv = (1.0 / np.sqrt(deg)).astype(np.float32)
    wtil = dinv[src] * edge_weight * dinv[dst]
    wself = dinv * dinv
    nodes = np.random.default_rng(12345).choice(N_NODES, 48, replace=False)
    W1f = np.asarray(W1, np.float32)
    b1f = np.asarray(b1, np.float32)
    W2f = np.asarray(W2, np.float32)
    b2f = np.asarray(b2, np.float32)

    z1 = np.zeros((N_PAD, D), np.float16)
    z1[:N_NODES] = x.astype(np.float16)
    for attempt in range(3):
        h1, res1 = run_layer(nc, z1, ixd, swd, cnt, W1, b1, trace=trace)
        if _spot_check(h1, z1, W1f, b1f, src, dst, wtil, wself, nodes):
            break
        print(f"[kernel] layer-1 spot check FAILED (attempt {attempt}); "
              "retrying launch")

    z2 = h1.astype(np.float16)
    for attempt in range(3):
        h2, res2 = run_layer(nc, z2, ixd, swd, cnt, W2, b2, trace=trace)
        if _spot_check(h2, z2, W2f, b2f, src, dst, wtil, wself, nodes):
            break
        print(f"[kernel] layer-2 spot check FAILED (attempt {attempt}); "
              "retrying launch")

    if trace:
        t1 = res1.exec_time_ns or 0
        t2 = res2.exec_time_ns or 0
        print(f"[kernel] layer1 exec: {t1} ns, layer2 exec: {t2} ns, "
              f"total: {t1 + t2} ns")
        kernel.last_exec_ns = t1 + t2
        kernel.last_results = (res1, res2)

    return h2[:N_NODES].astype(np.float32)

